# revision 47
# baseline (speedup 1.0000x reference)
"""CLCE loss kernel for Trainium2 (8 NeuronCores, SPMD).

Loss = 0.5 * cl + 0.5 * ce where
  cl_i = logsumexp(loss_temp_i) - slot0_i   over a [N, 2N-1] packed row
  ce   = cross-entropy of y_pred vs y_true.

Decomposition (exact, validated in f64 against the reference formula):
  cl_i = log(exp(slot0_i) + (T_i - P_i) + (2N-2 - num_neg_i)) - slot0_i
where
  T_i  = sum_j exp((xn_i . xn_j + 1) * 0.25)      <- the O(N^2 D) part, on device
  P_i  = sum_{j: y_j = y_i} exp(sim_ij)           <- O(N * class_size), on host
  slot0_i = sim_{i, first same-class j != i}      <- O(N), on host
  R_i  = sum_j exp(y_pred_ij)                     <- on device
  ce_i = log(R_i) - y_pred[i, y_i]

Device sharding: core c computes rows [512c, 512(c+1)) of the similarity
matrix as an fp8e4m3 DoubleRow matmul (2 MACs/cell/cycle; embeddings are
pre-scaled by S8 on the host so quantization error stays ~1e-4 relative on
each sim entry, which averages to ~1e-5 on the final scalar loss), with the
exp+row-sum fused into Scalar-engine activations (accum_out).  The
correction terms P_i/slot0_i are computed on the host in full precision
from the same normalized embeddings, so the handful of same-class entries
inside T_i cancel to fp8-noise level.
"""

import os
from contextlib import ExitStack

import numpy as np

import concourse.bass as bass
import concourse.tile as tile
from concourse import bacc, mybir
from concourse.bass_utils import run_bass_kernel_spmd

N, D, C = 4096, 1024, 512
TAU = 0.5
LAMBD = 0.5
NCORES = 8
BLK = N // NCORES          # 512 rows per core
P = 128                    # partitions
KT = D // 256              # 4 DoubleRow contraction super-tiles (256 each)
MT = BLK // P              # 4 output row tiles per core
W = 1024                   # column-chunk width (2 psum banks)
HC = N // W                # 4 column chunks
NS = W // 512              # matmuls per chunk k-step
S8 = 16.0                  # fp8 pre-scale for the embeddings

_F32 = mybir.dt.float32
_FP8 = mybir.dt.float8e4
_EXP = mybir.ActivationFunctionType.Exp
_DR = mybir.MatmulPerfMode.DoubleRow


def _build_kernel(tc, xt, wt, yp, out):
    """Emit the per-core Tile kernel.

    xt:  [KT*P, 2*N]   fp8  row kk*128+p, col i*N+n = S8*xn[n, kk*256+128i+p]
    wt:  [KT*P, 2*BLK] fp8  this core's column block, same packing
    yp:  [P, MT*C]     f32  this core's y_pred block, partition-major packed
    out: [P, MT*HC+MT] f32  T chunk-sums then R row-sums
    """
    nc = tc.nc
    with ExitStack() as ctx:
        pers = ctx.enter_context(tc.tile_pool(name="pers", bufs=1))
        epool = ctx.enter_context(tc.tile_pool(name="epool", bufs=2))
        psum = ctx.enter_context(
            tc.tile_pool(name="psum", bufs=3, space=bass.MemorySpace.PSUM)
        )

        # per-(kk, h) input tiles -> exact DMA->matmul dependencies.
        # The weights and the first column chunk arrive fused in one DMA per
        # kk (WX0) to halve the issue slots pacing the pipeline start.
        WX0 = [
            pers.tile([P, 2, BLK + W], _FP8, name=f"wx0_{k}", tag=f"wx0_{k}")
            for k in range(KT)
        ]
        XT = [
            [None] + [
                pers.tile([P, 2, W], _FP8, name=f"xtt{k}_{h}", tag=f"xtt{k}_{h}")
                for h in range(1, HC)
            ]
            for k in range(KT)
        ]
        WT = [WX0[k][:, :, 0:BLK] for k in range(KT)]
        for k in range(KT):
            XT[k][0] = WX0[k][:, :, BLK:BLK + W]
        YPB = pers.tile([P, MT * C], _F32)     # 8 KiB/partition
        # out layout: [Tparts (MT*HC) | Rparts (MT) | final-chunk 2nd half]
        OUTSB = pers.tile([P, MT * HC + MT + 1], _F32)
        bias_s = pers.tile([P, 1], _F32)       # 0.5*TAU for the sim affine
        bias_z = pers.tile([P, 1], _F32)       # 0.0 for plain exp
        warm = pers.tile([P, 1], _F32)

        ZW = pers.tile([P, 512], mybir.dt.bfloat16)  # zeros, PE warm-up operand

        nc.gpsimd.memset(ZW[:], 0.0)
        nc.gpsimd.memset(bias_s[:], 0.5 * TAU)
        nc.gpsimd.memset(bias_z[:], 0.0)
        # warm the exp table (ACT_TABLE_LOAD ~2.7us) before any data lands
        nc.scalar.activation(warm[:], bias_z[:], _EXP, bias=bias_z[:], scale=1.0)

        # PE warm-up: dummy matmuls spanning the input-DMA latency (~7us)
        # flip the HAM clock gate to 8/8 so the real stream starts at 2.4GHz
        wps = psum.tile([P, W], _F32, tag="ps")
        for _ in range(12):
            nc.tensor.matmul(wps[:, 0:512], ZW[:, 0:P], ZW[:], start=True, stop=True)

        # --- input DMAs.  Sync HWDGE carries the matmul operands in exactly
        # the order the PE consumes them: (WT kk, XT[kk][0]) pairs pace the
        # first chunk, then the later column chunks.  y_pred rides the
        # scalar HWDGE queue so it neither delays the sync stream nor the
        # CE activations. ---
        nc.scalar.dma_start(YPB[:], yp[:])
        xt3 = xt.rearrange("r (i n) -> r i n", i=2)
        wt3 = wt.rearrange("r (i n) -> r i n", i=2)
        for k in range(KT):
            nc.sync.dma_start(WX0[k][:], wt3[k * P:(k + 1) * P, :, :])
        for h in range(1, HC):
            for k in range(KT):
                nc.sync.dma_start(
                    XT[k][h][:],
                    xt3[k * P:(k + 1) * P, :, h * W:(h + 1) * W],
                )

        # --- CE: R[p, t] = sum_c exp(y_pred) ---
        for t in range(MT):
            et = epool.tile([P, W], _F32)
            nc.scalar.activation(
                et[:, 0:C], YPB[:, t * C:(t + 1) * C], _EXP,
                bias=bias_z[:], scale=1.0,
                accum_out=OUTSB[:, MT * HC + t:MT * HC + t + 1],
            )

        # --- main: sim block matmul + fused exp/row-sum ---
        # dot_scaled = S8^2 * xn_i . xn_j ; sim = (dot + 1) * 0.5 * TAU
        # -> exp(scale * dot_scaled + bias), scale = 0.5*TAU/S8^2, bias = 0.25
        act_scale = 0.5 * TAU / (S8 * S8)

        # first column chunk: k-outer over m=0..2 so the PE does three
        # m-tiles' work per arriving (WT k, XT k) pair -- stays dense behind
        # the DMA stream instead of stalling per k (which would re-throttle
        # the clock gate).  m=3 runs as a pipelined chunk afterward so its
        # matmuls cover the m=0..2 exp/row-sum drain and h=1 starts with a
        # free psum slot.
        ps_h0 = [
            psum.tile([P, W], _F32, tag="ps", name=f"psh0_{m}")
            for m in range(MT - 1)
        ]
        for k in range(KT):
            for m in range(MT - 1):
                for ns in range(NS):
                    nc.tensor.matmul(
                        ps_h0[m][:, ns * 512:(ns + 1) * 512],
                        WT[k][:, :, m * P:(m + 1) * P],
                        XT[k][0][:, :, ns * 512:(ns + 1) * 512],
                        start=(k == 0),
                        stop=(k == KT - 1),
                        perf_mode=_DR,
                    )
        for m in range(MT - 1):
            et = epool.tile([P, W], _F32)
            nc.scalar.activation(
                et[:], ps_h0[m][:], _EXP,
                bias=bias_s[:], scale=act_scale,
                accum_out=OUTSB[:, m * HC:m * HC + 1],
            )

        # remaining chunks: m-outer with psum-pool ping-pong (zero steady
        # state PE stalls; exp+row-sum runs concurrently on ScalarE)
        for h, m in [(0, MT - 1)] + [
            (h, m) for h in range(1, HC) for m in range(MT)
        ]:
            if h == HC - 1 and m == MT - 1:
                continue  # final chunk handled below
            ps = psum.tile([P, W], _F32, tag="ps")
            for k in range(KT):
                for ns in range(NS):
                    nc.tensor.matmul(
                        ps[:, ns * 512:(ns + 1) * 512],
                        WT[k][:, :, m * P:(m + 1) * P],
                        XT[k][h][:, :, ns * 512:(ns + 1) * 512],
                        start=(k == 0),
                        stop=(k == KT - 1),
                        perf_mode=_DR,
                    )
            et = epool.tile([P, W], _F32)
            nc.scalar.activation(
                et[:], ps[:], _EXP,
                bias=bias_s[:], scale=act_scale,
                accum_out=OUTSB[:, m * HC + h:m * HC + h + 1],
            )

        # final chunk (h=HC-1, m=MT-1): ns-outer over two dedicated 1-bank
        # psum tiles so the first half's exp/row-sum overlaps the second
        # half's matmuls -- the kernel tail is then only a [128, 512]
        # activation instead of [128, 1024]
        h, m = HC - 1, MT - 1
        for ns in range(NS):
            psl = psum.tile([P, 512], _F32, tag=f"pl{ns}", bufs=1,
                            name=f"psl{ns}")
            for k in range(KT):
                nc.tensor.matmul(
                    psl[:],
                    WT[k][:, :, m * P:(m + 1) * P],
                    XT[k][h][:, :, ns * 512:(ns + 1) * 512],
                    start=(k == 0),
                    stop=(k == KT - 1),
                    perf_mode=_DR,
                )
            et = epool.tile([P, W], _F32)
            col = m * HC + h if ns == 0 else MT * HC + MT
            nc.scalar.activation(
                et[:, 0:512], psl[:], _EXP,
                bias=bias_s[:], scale=act_scale,
                accum_out=OUTSB[:, col:col + 1],
            )

        nc.scalar.dma_start(out[:], OUTSB[:])


_NC_CACHE = None


def _get_nc():
    global _NC_CACHE
    if _NC_CACHE is None:
        nc = bacc.Bacc(
            "TRN2", target_bir_lowering=False, debug=False,
            enable_asserts=False, num_devices=NCORES,
        )
        xt_d = nc.dram_tensor("xt", [KT * P, 2 * N], _FP8, kind="ExternalInput")
        wt_d = nc.dram_tensor(
            "wt", [KT * P, 2 * (BLK + W)], _FP8, kind="ExternalInput"
        )
        yp_d = nc.dram_tensor("yp", [P, MT * C], _F32, kind="ExternalInput")
        out_d = nc.dram_tensor(
            "out", [P, MT * HC + MT + 1], _F32, kind="ExternalOutput"
        )
        with tile.TileContext(nc) as tc:
            _build_kernel(tc, xt_d.ap(), wt_d.ap(), yp_d.ap(), out_d.ap())
        nc.compile()
        _NC_CACHE = nc
    return _NC_CACHE


def _pack_fp8(zT, cols):
    """[D, ncols] f32 -> [KT*P, 2*ncols] fp8 with the DoubleRow pairing
    row kk*128+p, col i*ncols+n  <->  contraction index kk*256 + 128i + p."""
    fp8np = mybir.dt.np(_FP8)
    q = zT.reshape(KT, 2, P, cols).transpose(0, 2, 1, 3).reshape(KT * P, 2 * cols)
    return np.ascontiguousarray(q.astype(fp8np))


def _run_device(xnT, y_pred, trace=False):
    """Run the SPMD kernel; returns (T[N], R[N]) f64 and the raw results."""
    zT = (xnT * S8).astype(np.float32)  # [D, N], pre-scaled
    xt8 = _pack_fp8(zT, N)
    in_maps = []
    for c in range(NCORES):
        blk = slice(c * BLK, (c + 1) * BLK)
        ypb = (
            np.ascontiguousarray(y_pred[blk])
            .reshape(MT, P, C).transpose(1, 0, 2).reshape(P, MT * C)
        )
        wx0 = np.concatenate([zT[:, blk], zT[:, 0:W]], axis=1)
        in_maps.append({
            "xt": xt8,
            "wt": _pack_fp8(np.ascontiguousarray(wx0), BLK + W),
            "yp": np.ascontiguousarray(ypb),
        })
    res = run_bass_kernel_spmd(
        _get_nc(), in_maps, core_ids=list(range(NCORES)), trace=trace,
    )
    T = np.empty(N, np.float64)
    R = np.empty(N, np.float64)
    for c, r in enumerate(res.results):
        o = r["out"].astype(np.float64)  # [128, MT*HC + MT + 1]
        for m in range(MT):
            rows = slice(c * BLK + m * P, c * BLK + (m + 1) * P)
            T[rows] = o[:, m * HC:(m + 1) * HC].sum(axis=1)
            if m == MT - 1:
                T[rows] += o[:, MT * HC + MT]  # final chunk's 2nd half
            R[rows] = o[:, MT * HC + m]
    return T, R, res


def kernel(layer_embeds, y_true, y_pred):
    x = np.asarray(layer_embeds, dtype=np.float32)
    yt = np.asarray(y_true).astype(np.int64)
    yp = np.asarray(y_pred, dtype=np.float32)

    # normalize rows (torch-style eps clip)
    norms = np.maximum(
        np.sqrt((x.astype(np.float64) ** 2).sum(1, keepdims=True)), 1e-8
    )
    xn = (x / norms).astype(np.float32)
    xnT = np.ascontiguousarray(xn.T)  # [D, N]

    trace = bool(int(os.environ.get("CLCE_TRACE", "0")))
    T, R, res = _run_device(xnT, yp, trace=trace)
    if trace:
        kernel.last_results = res

    # --- host-side small terms (O(N * class_size)) ---
    # P_ must match what the device summed for the same-class entries, i.e.
    # the fp8-quantized sim values, so quantize the same way here.
    fp8np = mybir.dt.np(_FP8)
    xq = (xn * S8).astype(fp8np).astype(np.float64) / S8  # device-visible xn
    counts = np.bincount(yt, minlength=C)
    P_ = np.zeros(N, np.float64)
    slot0 = np.zeros(N, np.float64)
    for cval in np.unique(yt):
        idx = np.where(yt == cval)[0]
        subq = xq[idx]
        sq = (subq @ subq.T + 1.0) * (0.5 * TAU)   # device-matching sim
        P_[idx] = np.exp(sq).sum(1)
        if len(idx) >= 2:
            # slot0 feeds the final formula directly -> use full precision
            sub = xn[idx].astype(np.float64)
            s = (sub @ sub.T + 1.0) * (0.5 * TAU)
            firstpos = np.where(np.arange(len(idx)) == 0, 1, 0)
            slot0[idx] = s[np.arange(len(idx)), firstpos]

    num_neg = N - counts[yt]
    S = T - P_
    Z = (2 * N - 2 - num_neg).astype(np.float64)
    cl = (np.log(np.exp(slot0) + S + Z) - slot0).mean()
    ce = (
        np.log(R) - yp[np.arange(N), yt].astype(np.float64)
    ).mean()
    loss = LAMBD * cl + (1.0 - LAMBD) * ce
    return np.asarray(loss, dtype=np.float32)


# revision 52
# speedup vs baseline: 1.1314x; 1.1314x over previous
"""CLCE loss kernel for Trainium2 (8 NeuronCores, SPMD).

Loss = 0.5 * cl + 0.5 * ce where
  cl_i = logsumexp(loss_temp_i) - slot0_i   over a [N, 2N-1] packed row
  ce   = cross-entropy of y_pred vs y_true.

Decomposition (exact, validated in f64 against the reference formula):
  cl_i = log(exp(slot0_i) + (T_i - P_i) + (2N-2 - num_neg_i)) - slot0_i
where
  T_i  = sum_j exp((xn_i . xn_j + 1) * 0.25)      <- the O(N^2 D) part, on device
  P_i  = sum_{j: y_j = y_i} exp(sim_ij)           <- O(N * class_size), on host
  slot0_i = sim_{i, first same-class j != i}      <- O(N), on host
  R_i  = sum_j exp(y_pred_ij)                     <- on device
  ce_i = log(R_i) - y_pred[i, y_i]

Device sharding: core c computes rows [512c, 512(c+1)) of the similarity
matrix as an fp8e4m3 DoubleRow matmul (2 MACs/cell/cycle; embeddings are
pre-scaled by S8 on the host so quantization error stays ~1e-4 relative on
each sim entry, which averages to ~1e-5 on the final scalar loss), with the
exp+row-sum fused into Scalar-engine activations (accum_out).  The
correction terms P_i/slot0_i are computed on the host in full precision
from the same normalized embeddings, so the handful of same-class entries
inside T_i cancel to fp8-noise level.
"""

import os
from contextlib import ExitStack

import numpy as np

import concourse.bass as bass
import concourse.tile as tile
from concourse import bacc, mybir
from concourse.bass_utils import run_bass_kernel_spmd

N, D, C = 4096, 1024, 512
TAU = 0.5
LAMBD = 0.5
NCORES = 8
BLK = N // NCORES          # 512 rows per core
P = 128                    # partitions
KT = D // 256              # 4 DoubleRow contraction super-tiles (256 each)
MT = BLK // P              # 4 output row tiles per core
W = 1024                   # column-chunk width (2 psum banks)
HC = N // W                # 4 column chunks
NS = W // 512              # matmuls per chunk k-step
S8 = 16.0                  # fp8 pre-scale for the embeddings

_F32 = mybir.dt.float32
_FP8 = mybir.dt.float8e4
_EXP = mybir.ActivationFunctionType.Exp
_DR = mybir.MatmulPerfMode.DoubleRow


def _build_kernel(tc, xt, wt, yp, out):
    """Emit the per-core Tile kernel.

    xt:  [KT*P, 2*N]   fp8  row kk*128+p, col i*N+n = S8*xn[n, kk*256+128i+p]
    wt:  [KT*P, 2*BLK] fp8  this core's column block, same packing
    yp:  [P, MT*C]     f32  this core's y_pred block, partition-major packed
    out: [P, MT*HC+MT] f32  T chunk-sums then R row-sums
    """
    nc = tc.nc
    with ExitStack() as ctx:
        pers = ctx.enter_context(tc.tile_pool(name="pers", bufs=1))
        epool = ctx.enter_context(tc.tile_pool(name="epool", bufs=2))
        psum = ctx.enter_context(
            tc.tile_pool(name="psum", bufs=4, space=bass.MemorySpace.PSUM)
        )

        # per-(kk, h) input tiles -> exact DMA->matmul dependencies.
        # The weights and the first column chunk arrive fused in one DMA per
        # kk (WX0) to halve the issue slots pacing the pipeline start.
        WX0 = [
            pers.tile([P, 2, BLK + W], _FP8, name=f"wx0_{k}", tag=f"wx0_{k}")
            for k in range(KT)
        ]
        XT = [
            [None] + [
                pers.tile([P, 2, W], _FP8, name=f"xtt{k}_{h}", tag=f"xtt{k}_{h}")
                for h in range(1, HC)
            ]
            for k in range(KT)
        ]
        WT = [WX0[k][:, :, 0:BLK] for k in range(KT)]
        for k in range(KT):
            XT[k][0] = WX0[k][:, :, BLK:BLK + W]
        YPB = pers.tile([P, MT * C], _F32)     # 8 KiB/partition
        # out layout: [Tparts (MT*HC) | Rparts (MT)]
        OUTSB = pers.tile([P, MT * HC + MT], _F32)
        bias_s = pers.tile([P, 1], _F32)       # 0.5*TAU for the sim affine
        bias_z = pers.tile([P, 1], _F32)       # 0.0 for plain exp
        warm = pers.tile([P, 1], _F32)

        ZW = pers.tile([P, 512], mybir.dt.bfloat16)  # zeros, PE warm-up operand

        nc.gpsimd.memset(ZW[:], 0.0)
        nc.gpsimd.memset(bias_s[:], 0.5 * TAU)
        nc.gpsimd.memset(bias_z[:], 0.0)
        # warm the exp table (ACT_TABLE_LOAD ~2.7us) before any data lands
        nc.scalar.activation(warm[:], bias_z[:], _EXP, bias=bias_z[:], scale=1.0)

        # PE warm-up: dummy matmuls spanning the input-DMA latency (~7us)
        # flip the HAM clock gate to 8/8 so the real stream starts at 2.4GHz
        wps = psum.tile([P, W], _F32, tag="ps")
        for _ in range(12):
            nc.tensor.matmul(wps[:, 0:512], ZW[:, 0:P], ZW[:], start=True, stop=True)

        # --- input DMAs.  Sync HWDGE carries the matmul operands in exactly
        # the order the PE consumes them: (WT kk, XT[kk][0]) pairs pace the
        # first chunk, then the later column chunks.  y_pred rides the
        # scalar HWDGE queue so it neither delays the sync stream nor the
        # CE activations. ---
        nc.scalar.dma_start(YPB[:], yp[:])
        xt3 = xt.rearrange("r (i n) -> r i n", i=2)
        wt3 = wt.rearrange("r (i n) -> r i n", i=2)
        for k in range(KT):
            nc.sync.dma_start(WX0[k][:], wt3[k * P:(k + 1) * P, :, :])
        for h in range(1, HC):
            for k in range(KT):
                nc.sync.dma_start(
                    XT[k][h][:],
                    xt3[k * P:(k + 1) * P, :, h * W:(h + 1) * W],
                )

        # --- CE: R[p, t] = sum_c exp(y_pred) ---
        for t in range(MT):
            et = epool.tile([P, W], _F32)
            nc.scalar.activation(
                et[:, 0:C], YPB[:, t * C:(t + 1) * C], _EXP,
                bias=bias_z[:], scale=1.0,
                accum_out=OUTSB[:, MT * HC + t:MT * HC + t + 1],
            )

        # --- main: sim block matmul + fused exp/row-sum ---
        # dot_scaled = S8^2 * xn_i . xn_j ; sim = (dot + 1) * 0.5 * TAU
        # -> exp(scale * dot_scaled + bias), scale = 0.5*TAU/S8^2, bias = 0.25
        act_scale = 0.5 * TAU / (S8 * S8)

        # first column chunk: k-outer over m=0..2 so the PE does three
        # m-tiles' work per arriving (WT k, XT k) pair -- stays dense behind
        # the DMA stream instead of stalling per k (which would re-throttle
        # the clock gate).  m=3 runs as a pipelined chunk afterward so its
        # matmuls cover the m=0..2 exp/row-sum drain and h=1 starts with a
        # free psum slot.
        ps_h0 = [
            psum.tile([P, W], _F32, tag="ps", name=f"psh0_{m}")
            for m in range(MT - 1)
        ]
        for k in range(KT):
            for m in range(MT - 1):
                for ns in range(NS):
                    nc.tensor.matmul(
                        ps_h0[m][:, ns * 512:(ns + 1) * 512],
                        WT[k][:, :, m * P:(m + 1) * P],
                        XT[k][0][:, :, ns * 512:(ns + 1) * 512],
                        start=(k == 0),
                        stop=(k == KT - 1),
                        perf_mode=_DR,
                    )
        for m in range(MT - 1):
            et = epool.tile([P, W], _F32)
            nc.scalar.activation(
                et[:], ps_h0[m][:], _EXP,
                bias=bias_s[:], scale=act_scale,
                accum_out=OUTSB[:, m * HC:m * HC + 1],
            )

        # remaining chunks: m-outer with psum-pool ping-pong (zero steady
        # state PE stalls; exp+row-sum runs concurrently on ScalarE)
        for h, m in [(0, MT - 1)] + [
            (h, m) for h in range(1, HC) for m in range(MT)
        ]:
            if True:
                ps = psum.tile([P, W], _F32, tag="ps")
                for k in range(KT):
                    for ns in range(NS):
                        nc.tensor.matmul(
                            ps[:, ns * 512:(ns + 1) * 512],
                            WT[k][:, :, m * P:(m + 1) * P],
                            XT[k][h][:, :, ns * 512:(ns + 1) * 512],
                            start=(k == 0),
                            stop=(k == KT - 1),
                            perf_mode=_DR,
                        )
                et = epool.tile([P, W], _F32)
                nc.scalar.activation(
                    et[:], ps[:], _EXP,
                    bias=bias_s[:], scale=act_scale,
                    accum_out=OUTSB[:, m * HC + h:m * HC + h + 1],
                )

        nc.scalar.dma_start(out[:], OUTSB[:])


_NC_CACHE = None


def _get_nc():
    global _NC_CACHE
    if _NC_CACHE is None:
        nc = bacc.Bacc(
            "TRN2", target_bir_lowering=False, debug=False,
            enable_asserts=False, num_devices=NCORES,
        )
        xt_d = nc.dram_tensor("xt", [KT * P, 2 * N], _FP8, kind="ExternalInput")
        wt_d = nc.dram_tensor(
            "wt", [KT * P, 2 * (BLK + W)], _FP8, kind="ExternalInput"
        )
        yp_d = nc.dram_tensor("yp", [P, MT * C], _F32, kind="ExternalInput")
        out_d = nc.dram_tensor(
            "out", [P, MT * HC + MT], _F32, kind="ExternalOutput"
        )
        with tile.TileContext(nc) as tc:
            _build_kernel(tc, xt_d.ap(), wt_d.ap(), yp_d.ap(), out_d.ap())
        nc.compile()
        _NC_CACHE = nc
    return _NC_CACHE


def _pack_fp8(zT, cols):
    """[D, ncols] f32 -> [KT*P, 2*ncols] fp8 with the DoubleRow pairing
    row kk*128+p, col i*ncols+n  <->  contraction index kk*256 + 128i + p."""
    fp8np = mybir.dt.np(_FP8)
    q = zT.reshape(KT, 2, P, cols).transpose(0, 2, 1, 3).reshape(KT * P, 2 * cols)
    return np.ascontiguousarray(q.astype(fp8np))


def _run_device(xnT, y_pred, trace=False):
    """Run the SPMD kernel; returns (T[N], R[N]) f64 and the raw results."""
    zT = (xnT * S8).astype(np.float32)  # [D, N], pre-scaled
    xt8 = _pack_fp8(zT, N)
    in_maps = []
    for c in range(NCORES):
        blk = slice(c * BLK, (c + 1) * BLK)
        ypb = (
            np.ascontiguousarray(y_pred[blk])
            .reshape(MT, P, C).transpose(1, 0, 2).reshape(P, MT * C)
        )
        wx0 = np.concatenate([zT[:, blk], zT[:, 0:W]], axis=1)
        in_maps.append({
            "xt": xt8,
            "wt": _pack_fp8(np.ascontiguousarray(wx0), BLK + W),
            "yp": np.ascontiguousarray(ypb),
        })
    res = run_bass_kernel_spmd(
        _get_nc(), in_maps, core_ids=list(range(NCORES)), trace=trace,
    )
    T = np.empty(N, np.float64)
    R = np.empty(N, np.float64)
    for c, r in enumerate(res.results):
        o = r["out"].astype(np.float64)  # [128, MT*HC + MT]
        for m in range(MT):
            rows = slice(c * BLK + m * P, c * BLK + (m + 1) * P)
            T[rows] = o[:, m * HC:(m + 1) * HC].sum(axis=1)
            R[rows] = o[:, MT * HC + m]
    return T, R, res


def kernel(layer_embeds, y_true, y_pred):
    x = np.asarray(layer_embeds, dtype=np.float32)
    yt = np.asarray(y_true).astype(np.int64)
    yp = np.asarray(y_pred, dtype=np.float32)

    # normalize rows (torch-style eps clip)
    norms = np.maximum(
        np.sqrt((x.astype(np.float64) ** 2).sum(1, keepdims=True)), 1e-8
    )
    xn = (x / norms).astype(np.float32)
    xnT = np.ascontiguousarray(xn.T)  # [D, N]

    trace = bool(int(os.environ.get("CLCE_TRACE", "0")))
    T, R, res = _run_device(xnT, yp, trace=trace)
    if trace:
        kernel.last_results = res

    # --- host-side small terms (O(N * class_size)) ---
    # P_ must match what the device summed for the same-class entries, i.e.
    # the fp8-quantized sim values, so quantize the same way here.
    fp8np = mybir.dt.np(_FP8)
    xq = (xn * S8).astype(fp8np).astype(np.float64) / S8  # device-visible xn
    counts = np.bincount(yt, minlength=C)
    P_ = np.zeros(N, np.float64)
    slot0 = np.zeros(N, np.float64)
    for cval in np.unique(yt):
        idx = np.where(yt == cval)[0]
        subq = xq[idx]
        sq = (subq @ subq.T + 1.0) * (0.5 * TAU)   # device-matching sim
        P_[idx] = np.exp(sq).sum(1)
        if len(idx) >= 2:
            # slot0 feeds the final formula directly -> use full precision
            sub = xn[idx].astype(np.float64)
            s = (sub @ sub.T + 1.0) * (0.5 * TAU)
            firstpos = np.where(np.arange(len(idx)) == 0, 1, 0)
            slot0[idx] = s[np.arange(len(idx)), firstpos]

    num_neg = N - counts[yt]
    S = T - P_
    Z = (2 * N - 2 - num_neg).astype(np.float64)
    cl = (np.log(np.exp(slot0) + S + Z) - slot0).mean()
    ce = (
        np.log(R) - yp[np.arange(N), yt].astype(np.float64)
    ).mean()
    loss = LAMBD * cl + (1.0 - LAMBD) * ce
    return np.asarray(loss, dtype=np.float32)
